# revision 1
# baseline (speedup 1.0000x reference)
"""ChannelAttention kernel for Trainium2 (8 NeuronCores, batch-parallel).

Reference computation per batch element b (C=64, N=H*W=65536):
    X1 = x[b] viewed [C, N]          (proj_query)
    X2 = x[b] viewed [N, C]          (proj_key -- a reshape, NOT a transpose)
    S  = X1 @ X2                     [C, C]
    P  = softmax(S, axis=-1)
    out[b] = (P @ X1) + X1  =  (P + I) @ X1

Sharding: data-parallel over batch. B=16 -> 2 batches per core on 8 cores.

Per-core dataflow (per batch):
  - H strips [128, 2048] f32: partition h*64+c holds X1[c, half-h window].
    Rolling pool (transposes consume them in order); each strip is also
    cast to a persistent bf16 copy on GPSIMD (mm2's rhs).
  - mm1 rhs: U2 tiles = contiguous 64KB chunks of x[b] viewed [128, 128]:
    U2_t[p, 64e+d] = X2[t*256+2p+e, d].  512B/partition contiguous -> full
    DMA rate (the old [128,·,64] layout was 256B grains = half rate).
  - mm1 lhsT: PE-transposes of stride-2 strip slices -> partition i holds
    n = k*256+2i(+1) for both column halves; even/odd matmuls pair with
    U2 column slices 0:64 / 64:128.  Diagonal 64x64 blocks of the [128,128]
    accumulator hold half0/half1 contributions; S = UL + LR.
  - softmax: DVE row-max (negated) -> ACT exp with fused row-sum ->
    DVE reciprocal -> fused (E * 1/sum) + I -> PE transpose -> bf16.
  - mm2 in bf16 (1 cyc/row on PE vs 4 for f32): (P+I)^T bf16 @ bf16 strips
    -> PSUM f32 -> bf16 staging -> bf16 stores (host upcasts to f32).
"""

import numpy as np

_CACHE = {}

B_FULL = 16
C = 64
N = 65536          # H*W = 256*256
NB = 2             # batches per core
NCORES = 8
NSTRIP = 16        # strips per batch; strip = [128, 2048] (both halves)
STRIPW = 2048
NUNIT = 128        # 256-wide n units per half (32768 / 256)
UPS = 8            # units per strip
SKEW = 2           # transpose-emission skew (unit pairs)


def _build(loop_reps=1):
    from contextlib import nullcontext

    import concourse.bacc as bacc
    import concourse.mybir as mybir
    import concourse.tile as tile
    from concourse.masks import make_identity

    f32 = mybir.dt.float32
    bf16 = mybir.dt.bfloat16
    Alu = mybir.AluOpType
    Act = mybir.ActivationFunctionType

    nc = bacc.Bacc("TRN2", debug=False)
    xb = nc.dram_tensor("xb", [NB, C * N], f32, kind="ExternalInput").ap()
    ob = nc.dram_tensor("ob", [NB, C * N], bf16, kind="ExternalOutput").ap()

    with tile.TileContext(nc) as tc:
        with (
            tc.tile_pool(name="consts", bufs=1) as consts,
            tc.tile_pool(name="Hf", bufs=10) as hpoolf,
            tc.tile_pool(name="Hb", bufs=16) as hpoolb,
            tc.tile_pool(name="V", bufs=4) as vpool,
            tc.tile_pool(name="TOs", bufs=4) as topool,
            tc.tile_pool(name="stage", bufs=3) as stpool,
            tc.tile_pool(name="soft", bufs=2) as softpool,
            tc.tile_pool(name="psT", bufs=3, space="PSUM") as psT,
            tc.tile_pool(name="psS", bufs=1, space="PSUM") as psS,
            tc.tile_pool(name="psO", bufs=3, space="PSUM") as psO,
            tc.tile_pool(name="psP", bufs=1, space="PSUM") as psP,
        ):
            ident = consts.tile([128, 128], f32)
            make_identity(nc, ident[:])

            state = {}         # per-batch tile lists

            def emit_loads(b):
                x1 = xb[b].rearrange("(c n) -> c n", c=C)      # [64, 65536]
                stripsf, stripsb, vtiles = [], [], []
                for j in range(NSTRIP):
                    stf = hpoolf.tile([128, STRIPW], f32, tag="Hf")
                    nc.sync.dma_start(
                        stf[0:64, :], x1[:, j * STRIPW:(j + 1) * STRIPW]
                    )
                    nc.sync.dma_start(
                        stf[64:128, :],
                        x1[:, 32768 + j * STRIPW:32768 + (j + 1) * STRIPW],
                    )
                    stb = hpoolb.tile([128, STRIPW], bf16, tag="Hb")
                    nc.gpsimd.tensor_copy(stb[:], stf[:])
                    stripsf.append(stf)
                    stripsb.append(stb)
                    # U2 chunk j: tiles t = 8j+tl (half0), 128+8j+tl (half1)
                    vt = vpool.tile([128, UPS, 2, 128], f32, tag="V")
                    for half in range(2):
                        t0 = 128 * half + UPS * j
                        src = xb[b][t0 * 16384:(t0 + UPS) * 16384].rearrange(
                            "(t p c) -> p t c", p=128, c=128
                        )
                        nc.scalar.dma_start(vt[:, :, half, :], src)
                    vtiles.append(vt)
                state[b] = (stripsf, stripsb, vtiles)

            def emit_mm1_softmax(b):
                # mm1: S accumulated over 64 unit-pairs x (even, odd).
                # Unit pair p covers units 2p, 2p+1: 4 transposes into one
                # PSUM bank, one batched copy, then 4 accumulating matmuls.
                stripsf, stripsb, vtiles = state[b]
                s_ps = psS.tile([128, 128], f32, tag="S")
                to_sbs = {}
                NPAIR = NUNIT // 2

                def emit_transpose(pp):
                    to_ps = psT.tile([128, 4, 128], f32, tag="TO")
                    for q in range(2):
                        k = 2 * pp + q
                        j, tl = k // UPS, k % UPS
                        stv = stripsf[j].rearrange(
                            "p (a two) -> p a two", two=2
                        )
                        for ph in range(2):
                            nc.tensor.transpose(
                                to_ps[:, 2 * q + ph, :],
                                stv[:, tl * 128:(tl + 1) * 128, ph],
                                ident[:],
                            )
                    to_sb = topool.tile([128, 4, 128], f32, tag="TOs")
                    if pp % 2 == 0:
                        nc.scalar.copy(to_sb[:], to_ps[:])
                    else:
                        nc.vector.tensor_copy(to_sb[:], to_ps[:])
                    to_sbs[pp] = to_sb

                for pp in range(SKEW):
                    emit_transpose(pp)
                for pp in range(NPAIR):
                    if pp + SKEW < NPAIR:
                        emit_transpose(pp + SKEW)
                    to_sb = to_sbs.pop(pp)
                    for q in range(2):
                        k = 2 * pp + q
                        j, tl = k // UPS, k % UPS
                        for ph in range(2):
                            nc.tensor.matmul(
                                s_ps[:], to_sb[:, 2 * q + ph, :],
                                vtiles[j][:, tl, :, 64 * ph:64 * ph + 64],
                                start=(k == 0 and ph == 0),
                                stop=(k == NUNIT - 1 and ph == 1),
                            )

                # S = UL + LR (diagonal blocks of the accumulator)
                s_sb = softpool.tile([128, 128], f32, tag="Ssb")
                nc.vector.tensor_copy(s_sb[:], s_ps[:])
                s_fix = softpool.tile([64, 64], f32, tag="Sfix")
                nc.sync.dma_start(s_fix[:], s_sb[64:128, 64:128])
                s2_sb = softpool.tile([64, 64], f32, tag="S2")
                nc.vector.tensor_add(s2_sb[:], s_sb[0:64, 0:64], s_fix[:])

                # softmax + (P + I), transposed, cast to bf16
                nmx = softpool.tile([64, 1], f32, tag="nmx")
                nc.vector.tensor_reduce(
                    nmx[:], s2_sb[:], axis=mybir.AxisListType.X, op=Alu.max,
                    negate=True,
                )
                esum = softpool.tile([64, 1], f32, tag="esum")
                e_sb = softpool.tile([64, 64], f32, tag="E")
                nc.scalar.activation(
                    e_sb[:], s2_sb[:], Act.Exp, bias=nmx[:, 0:1], scale=1.0,
                    accum_out=esum[:],
                )
                rcp = softpool.tile([64, 1], f32, tag="rcp")
                nc.vector.reciprocal(rcp[:], esum[:])
                pi_sb = softpool.tile([64, 64], f32, tag="PI")
                # PI = (E * 1/sum) + I
                nc.vector.scalar_tensor_tensor(
                    pi_sb[:], e_sb[:], rcp[:, 0:1], ident[0:64, 0:64],
                    Alu.mult, Alu.add,
                )
                pit_ps = psP.tile([64, 64], f32, tag="PIT")
                nc.tensor.transpose(pit_ps[:], pi_sb[:], ident[0:64, 0:64])
                pit = softpool.tile([128, 64], bf16, tag="PITb")
                nc.vector.tensor_copy(pit[0:64, :], pit_ps[:])
                nc.sync.dma_start(pit[64:128, :], pit[0:64, :])
                return pit

            def emit_mm2(b, pit):
                # mm2 (bf16): out = (P+I) @ X1, 64 windows of 512 per half
                stripsf, stripsb, vtiles = state[b]
                o1 = ob[b].rearrange("(c n) -> c n", c=C)
                for half in range(2):
                    lhs = pit[64 * half:64 * half + 64, :]
                    for g in range(8):            # groups of 8 windows (4096)
                        stg = stpool.tile([128, 4, 512], bf16, tag="stage")
                        for wi in range(4):
                            o_ps = psO.tile([128, 512], f32, tag="O")
                            for hb in range(2):
                                stb = stripsb[2 * g + hb]
                                rhs = stb[64 * half:64 * half + 64,
                                          wi * 512:wi * 512 + 512]
                                nc.tensor.matmul(
                                    o_ps[64 * hb:64 * hb + 64, :], lhs, rhs,
                                    start=True, stop=True,
                                )
                            if wi % 2 == 0:
                                nc.vector.tensor_copy(
                                    stg[:, wi, :], o_ps[:]
                                )
                            else:
                                nc.scalar.copy(
                                    stg[:, wi, :], o_ps[:]
                                )
                        off = 32768 * half + g * 4096
                        nc.scalar.dma_start(
                            o1[:, off:off + 2048],
                            stg[0:64].rearrange("p a b -> p (a b)"),
                        )
                        nc.sync.dma_start(
                            o1[:, off + 2048:off + 4096],
                            stg[64:128].rearrange("p a b -> p (a b)"),
                        )
                del state[b]

            loop_cm = (
                tc.For_i(0, loop_reps, 1) if loop_reps > 1 else nullcontext()
            )
            with loop_cm:
                # Software-pipelined emission: batch b+1's load DMAs are
                # enqueued on the rings BEFORE batch b's mm2/stores so the
                # DMA engines prefetch the next batch during mm2.
                emit_loads(0)
                for b in range(NB):
                    pit = emit_mm1_softmax(b)
                    if b + 1 < NB:
                        emit_loads(b + 1)
                    emit_mm2(b, pit)

    nc.compile()
    return nc


def kernel(x: np.ndarray) -> np.ndarray:
    import ml_dtypes
    from concourse.bass_utils import run_bass_kernel_spmd

    if "nc" not in _CACHE:
        _CACHE["nc"] = _build()
    nc = _CACHE["nc"]

    x = np.ascontiguousarray(x, dtype=np.float32)
    B, Cc, H, W = x.shape
    xflat = x.reshape(B, Cc * H * W)
    in_maps = [
        {"xb": xflat[NB * i:NB * (i + 1)]} for i in range(NCORES)
    ]
    res = run_bass_kernel_spmd(nc, in_maps, core_ids=list(range(NCORES)))
    out = np.empty_like(xflat)
    for i in range(NCORES):
        ob = res.results[i]["ob"]
        out[NB * i:NB * (i + 1)] = ob.view(ml_dtypes.bfloat16).astype(
            np.float32
        ) if ob.dtype != ml_dtypes.bfloat16 else ob.astype(np.float32)
    return out.reshape(B, Cc, H, W)



# revision 2
# speedup vs baseline: 27.9166x; 27.9166x over previous
"""ChannelAttention kernel for Trainium2 (8 NeuronCores, batch-parallel).

Reference computation per batch element b (C=64, N=H*W=65536):
    X1 = x[b] viewed [C, N]          (proj_query)
    X2 = x[b] viewed [N, C]          (proj_key -- a reshape, NOT a transpose)
    S  = X1 @ X2                     [C, C]
    P  = softmax(S, axis=-1)
    out[b] = (P @ X1) + X1  =  (P + I) @ X1

Sharding: data-parallel over batch. B=16 -> 2 batches per core on 8 cores.

Per-core dataflow (per batch):
  - H strips [128, 2048] f32: partition h*64+c holds X1[c, half-h window].
    Rolling pool (transposes consume them in order); each strip is also
    cast to a persistent bf16 copy on GPSIMD (mm2's rhs).
  - mm1 rhs: U2 tiles = contiguous 64KB chunks of x[b] viewed [128, 128]:
    U2_t[p, 64e+d] = X2[t*256+2p+e, d].  512B/partition contiguous -> full
    DMA rate (the old [128,·,64] layout was 256B grains = half rate).
  - mm1 lhsT: PE-transposes of stride-2 strip slices -> partition i holds
    n = k*256+2i(+1) for both column halves; even/odd matmuls pair with
    U2 column slices 0:64 / 64:128.  Diagonal 64x64 blocks of the [128,128]
    accumulator hold half0/half1 contributions; S = UL + LR.
  - softmax: DVE row-max (negated) -> ACT exp with fused row-sum ->
    DVE reciprocal -> fused (E * 1/sum) + I -> PE transpose -> bf16.
  - mm2 in bf16 (1 cyc/row on PE vs 4 for f32): (P+I)^T bf16 @ bf16 strips
    -> PSUM f32 -> bf16 staging -> bf16 stores (host upcasts to f32).
"""

import numpy as np

_CACHE = {}

B_FULL = 16
C = 64
N = 65536          # H*W = 256*256
NB = 2             # batches per core
NCORES = 8
NSTRIP = 16        # strips per batch; strip = [128, 2048] (both halves)
STRIPW = 2048
NUNIT = 128        # 256-wide n units per half (32768 / 256)
UPS = 8            # units per strip
SKEW = 2           # transpose-emission skew (unit pairs)


def _build(loop_reps=1):
    from contextlib import nullcontext

    import concourse.bacc as bacc
    import concourse.mybir as mybir
    import concourse.tile as tile
    from concourse.masks import make_identity

    f32 = mybir.dt.float32
    bf16 = mybir.dt.bfloat16
    Alu = mybir.AluOpType
    Act = mybir.ActivationFunctionType

    nc = bacc.Bacc("TRN2", debug=False)
    xb = nc.dram_tensor("xb", [NB, C * N], f32, kind="ExternalInput").ap()
    ob = nc.dram_tensor("ob", [NB, C * N], bf16, kind="ExternalOutput").ap()

    with tile.TileContext(nc) as tc:
        with (
            tc.tile_pool(name="consts", bufs=1) as consts,
            tc.tile_pool(name="Hf", bufs=10) as hpoolf,
            tc.tile_pool(name="Hb", bufs=16) as hpoolb,
            tc.tile_pool(name="V", bufs=4) as vpool,
            tc.tile_pool(name="TOs", bufs=4) as topool,
            tc.tile_pool(name="stage", bufs=3) as stpool,
            tc.tile_pool(name="soft", bufs=2) as softpool,
            tc.tile_pool(name="psT", bufs=3, space="PSUM") as psT,
            tc.tile_pool(name="psS", bufs=1, space="PSUM") as psS,
            tc.tile_pool(name="psO", bufs=3, space="PSUM") as psO,
            tc.tile_pool(name="psP", bufs=1, space="PSUM") as psP,
        ):
            ident = consts.tile([128, 128], f32)
            make_identity(nc, ident[:])

            state = {}         # per-batch tile lists

            def emit_loads(b):
                x1 = xb[b].rearrange("(c n) -> c n", c=C)      # [64, 65536]
                stripsf, stripsb, vtiles = [], [], []
                for j in range(NSTRIP):
                    stf = hpoolf.tile([128, STRIPW], f32, tag="Hf")
                    nc.sync.dma_start(
                        stf[0:64, :], x1[:, j * STRIPW:(j + 1) * STRIPW]
                    )
                    nc.sync.dma_start(
                        stf[64:128, :],
                        x1[:, 32768 + j * STRIPW:32768 + (j + 1) * STRIPW],
                    )
                    stb = hpoolb.tile([128, STRIPW], bf16, tag="Hb")
                    nc.gpsimd.tensor_copy(stb[:], stf[:])
                    stripsf.append(stf)
                    stripsb.append(stb)
                    # U2 chunk j: tiles t = 8j+tl (half0), 128+8j+tl (half1)
                    vt = vpool.tile([128, UPS, 2, 128], f32, tag="V")
                    for half in range(2):
                        t0 = 128 * half + UPS * j
                        src = xb[b][t0 * 16384:(t0 + UPS) * 16384].rearrange(
                            "(t p c) -> p t c", p=128, c=128
                        )
                        nc.scalar.dma_start(vt[:, :, half, :], src)
                    vtiles.append(vt)
                state[b] = (stripsf, stripsb, vtiles)

            def emit_mm1_softmax(b):
                # mm1: S accumulated over 64 unit-pairs x (even, odd).
                # Unit pair p covers units 2p, 2p+1: 4 transposes into one
                # PSUM bank, one batched copy, then 4 accumulating matmuls.
                stripsf, stripsb, vtiles = state[b]
                s_ps = psS.tile([128, 128], f32, tag="S")
                to_sbs = {}
                NPAIR = NUNIT // 2

                def emit_transpose(pp):
                    to_ps = psT.tile([128, 4, 128], f32, tag="TO")
                    for q in range(2):
                        k = 2 * pp + q
                        j, tl = k // UPS, k % UPS
                        stv = stripsf[j].rearrange(
                            "p (a two) -> p a two", two=2
                        )
                        for ph in range(2):
                            nc.tensor.transpose(
                                to_ps[:, 2 * q + ph, :],
                                stv[:, tl * 128:(tl + 1) * 128, ph],
                                ident[:],
                            )
                    to_sb = topool.tile([128, 4, 128], f32, tag="TOs")
                    if pp % 2 == 0:
                        nc.scalar.copy(to_sb[:], to_ps[:])
                    else:
                        nc.vector.tensor_copy(to_sb[:], to_ps[:])
                    to_sbs[pp] = to_sb

                for pp in range(SKEW):
                    emit_transpose(pp)
                for pp in range(NPAIR):
                    if pp + SKEW < NPAIR:
                        emit_transpose(pp + SKEW)
                    to_sb = to_sbs.pop(pp)
                    for q in range(2):
                        k = 2 * pp + q
                        j, tl = k // UPS, k % UPS
                        for ph in range(2):
                            nc.tensor.matmul(
                                s_ps[:], to_sb[:, 2 * q + ph, :],
                                vtiles[j][:, tl, :, 64 * ph:64 * ph + 64],
                                start=(k == 0 and ph == 0),
                                stop=(k == NUNIT - 1 and ph == 1),
                            )

                # S = UL + LR (diagonal blocks of the accumulator)
                s_sb = softpool.tile([128, 128], f32, tag="Ssb")
                nc.vector.tensor_copy(s_sb[:], s_ps[:])
                s_fix = softpool.tile([64, 64], f32, tag="Sfix")
                nc.sync.dma_start(s_fix[:], s_sb[64:128, 64:128])
                s2_sb = softpool.tile([64, 64], f32, tag="S2")
                nc.vector.tensor_add(s2_sb[:], s_sb[0:64, 0:64], s_fix[:])

                # softmax + (P + I), transposed, cast to bf16
                nmx = softpool.tile([64, 1], f32, tag="nmx")
                nc.vector.tensor_reduce(
                    nmx[:], s2_sb[:], axis=mybir.AxisListType.X, op=Alu.max,
                    negate=True,
                )
                esum = softpool.tile([64, 1], f32, tag="esum")
                e_sb = softpool.tile([64, 64], f32, tag="E")
                nc.scalar.activation(
                    e_sb[:], s2_sb[:], Act.Exp, bias=nmx[:, 0:1], scale=1.0,
                    accum_out=esum[:],
                )
                rcp = softpool.tile([64, 1], f32, tag="rcp")
                nc.vector.reciprocal(rcp[:], esum[:])
                pi_sb = softpool.tile([64, 64], f32, tag="PI")
                # PI = (E * 1/sum) + I
                nc.vector.scalar_tensor_tensor(
                    pi_sb[:], e_sb[:], rcp[:, 0:1], ident[0:64, 0:64],
                    Alu.mult, Alu.add,
                )
                pit_ps = psP.tile([64, 64], f32, tag="PIT")
                nc.tensor.transpose(pit_ps[:], pi_sb[:], ident[0:64, 0:64])
                pit = softpool.tile([128, 64], bf16, tag="PITb")
                nc.vector.tensor_copy(pit[0:64, :], pit_ps[:])
                nc.sync.dma_start(pit[64:128, :], pit[0:64, :])
                return pit

            def emit_mm2(b, pit):
                # mm2 (bf16): out = (P+I) @ X1, 64 windows of 512 per half
                stripsf, stripsb, vtiles = state[b]
                o1 = ob[b].rearrange("(c n) -> c n", c=C)
                for half in range(2):
                    lhs = pit[64 * half:64 * half + 64, :]
                    for g in range(8):            # groups of 8 windows (4096)
                        stg = stpool.tile([128, 4, 512], bf16, tag="stage")
                        for wi in range(4):
                            o_ps = psO.tile([128, 512], f32, tag="O")
                            for hb in range(2):
                                stb = stripsb[2 * g + hb]
                                rhs = stb[64 * half:64 * half + 64,
                                          wi * 512:wi * 512 + 512]
                                nc.tensor.matmul(
                                    o_ps[64 * hb:64 * hb + 64, :], lhs, rhs,
                                    start=True, stop=True,
                                )
                            if wi % 2 == 0:
                                nc.vector.tensor_copy(
                                    stg[:, wi, :], o_ps[:]
                                )
                            else:
                                nc.scalar.copy(
                                    stg[:, wi, :], o_ps[:]
                                )
                        off = 32768 * half + g * 4096
                        nc.scalar.dma_start(
                            o1[:, off:off + 2048],
                            stg[0:64].rearrange("p a b -> p (a b)"),
                        )
                        nc.sync.dma_start(
                            o1[:, off + 2048:off + 4096],
                            stg[64:128].rearrange("p a b -> p (a b)"),
                        )
                del state[b]

            loop_cm = (
                tc.For_i(0, loop_reps, 1) if loop_reps > 1 else nullcontext()
            )
            with loop_cm:
                # Software-pipelined emission: batch b+1's load DMAs are
                # enqueued on the rings BEFORE batch b's mm2/stores so the
                # DMA engines prefetch the next batch during mm2.
                emit_loads(0)
                for b in range(NB):
                    pit = emit_mm1_softmax(b)
                    if b + 1 < NB:
                        emit_loads(b + 1)
                    emit_mm2(b, pit)

    nc.compile()
    return nc


def make_in_maps(x: np.ndarray) -> list:
    x = np.ascontiguousarray(x, dtype=np.float32)
    xflat = x.reshape(x.shape[0], -1)
    return [{"xb": xflat[NB * i:NB * (i + 1)]} for i in range(NCORES)]


def kernel(x: np.ndarray) -> np.ndarray:
    import ml_dtypes
    from concourse.bass_utils import run_bass_kernel_spmd

    if "nc" not in _CACHE:
        _CACHE["nc"] = _build()
    nc = _CACHE["nc"]

    x = np.ascontiguousarray(x, dtype=np.float32)
    B, Cc, H, W = x.shape
    xflat = x.reshape(B, Cc * H * W)
    in_maps = make_in_maps(x)
    res = run_bass_kernel_spmd(nc, in_maps, core_ids=list(range(NCORES)))
    out = np.empty_like(xflat)
    for i in range(NCORES):
        ob = res.results[i]["ob"]
        out[NB * i:NB * (i + 1)] = ob.view(ml_dtypes.bfloat16).astype(
            np.float32
        ) if ob.dtype != ml_dtypes.bfloat16 else ob.astype(np.float32)
    return out.reshape(B, Cc, H, W)



# revision 3
# speedup vs baseline: 29.6893x; 1.0635x over previous
"""ChannelAttention kernel for Trainium2 (8 NeuronCores, batch-parallel).

Reference computation per batch element b (C=64, N=H*W=65536):
    X1 = x[b] viewed [C, N]          (proj_query)
    X2 = x[b] viewed [N, C]          (proj_key -- a reshape, NOT a transpose)
    S  = X1 @ X2                     [C, C]
    P  = softmax(S, axis=-1)
    out[b] = (P @ X1) + X1  =  (P + I) @ X1

Sharding: data-parallel over batch. B=16 -> 2 batches per core on 8 cores.

Key scheduling decisions (HW-measured):
  - mm1 split into per-half [128 x 64 x 64] matmuls accumulating both
    n-halves directly into one [64,64] PSUM block: removes the s_fix
    partition-shift DMA + s_sb copy + add from the softmax critical path.
  - softmax reads S straight from PSUM.
  - Output stores go through SWDGE (gpsimd) so they never queue behind
    next-batch load DMAs on the HWDGE FIFO rings (the main mm2 stall in
    v1: stores sat behind ~94us of queued loads, exhausting the staging
    pool).  Pool FIFO order per batch is [stores(b), casts(b+1)].
  - SKEW=4 with topool=6 (deeper transpose lookahead).
"""

import numpy as np

_CACHE = {}

B_FULL = 16
C = 64
N = 65536          # H*W = 256*256
NB = 2             # batches per core
NCORES = 8
NSTRIP = 16        # strips per batch; strip = [128, 2048] (both halves)
STRIPW = 2048
NUNIT = 128        # 256-wide n units per half (32768 / 256)
UPS = 8            # units per strip
SKEW = 4           # transpose-emission skew (unit pairs)


def _build(loop_reps=1):
    from contextlib import nullcontext

    import concourse.bacc as bacc
    import concourse.mybir as mybir
    import concourse.tile as tile
    from concourse.masks import make_identity

    f32 = mybir.dt.float32
    bf16 = mybir.dt.bfloat16
    Alu = mybir.AluOpType
    Act = mybir.ActivationFunctionType

    nc = bacc.Bacc("TRN2", debug=False)
    xb = nc.dram_tensor("xb", [NB, C * N], f32, kind="ExternalInput").ap()
    ob = nc.dram_tensor("ob", [NB, C * N], bf16, kind="ExternalOutput").ap()

    with tile.TileContext(nc) as tc:
        with (
            tc.tile_pool(name="consts", bufs=1) as consts,
            tc.tile_pool(name="Hf", bufs=10) as hpoolf,
            tc.tile_pool(name="Hb", bufs=16) as hpoolb,
            tc.tile_pool(name="V", bufs=4) as vpool,
            tc.tile_pool(name="TOs", bufs=6) as topool,
            tc.tile_pool(name="stage", bufs=3) as stpool,
            tc.tile_pool(name="soft", bufs=2) as softpool,
            tc.tile_pool(name="psT", bufs=3, space="PSUM") as psT,
            tc.tile_pool(name="psS", bufs=1, space="PSUM") as psS,
            tc.tile_pool(name="psO", bufs=3, space="PSUM") as psO,
            tc.tile_pool(name="psP", bufs=1, space="PSUM") as psP,
        ):
            ident = consts.tile([128, 128], f32)
            make_identity(nc, ident[:])

            state = {}         # per-batch tile lists

            def emit_loads(b):
                x1 = xb[b].rearrange("(c n) -> c n", c=C)      # [64, 65536]
                stripsf, stripsb, vtiles = [], [], []
                state[b] = (stripsf, stripsb, vtiles)
                for j in range(NSTRIP):
                    stf = hpoolf.tile([128, STRIPW], f32, tag="Hf")
                    nc.sync.dma_start(
                        stf[0:64, :], x1[:, j * STRIPW:(j + 1) * STRIPW]
                    )
                    nc.sync.dma_start(
                        stf[64:128, :],
                        x1[:, 32768 + j * STRIPW:32768 + (j + 1) * STRIPW],
                    )
                    stripsf.append(stf)
                    # U2 chunk j: tiles t = 8j+tl (half0), 128+8j+tl (half1)
                    vt = vpool.tile([128, UPS, 2, 128], f32, tag="V")
                    for half in range(2):
                        t0 = 128 * half + UPS * j
                        src = xb[b][t0 * 16384:(t0 + UPS) * 16384].rearrange(
                            "(t p c) -> p t c", p=128, c=128
                        )
                        nc.scalar.dma_start(vt[:, :, half, :], src)
                    vtiles.append(vt)

            def emit_casts(b):
                # bf16 casts for mm2's rhs.  Emitted AFTER the previous
                # batch's stores so the Pool FIFO order is
                # [stores(b-1), casts(b)] -- stores are ready first, casts
                # are only consumed by mm2(b) much later.
                stripsf, stripsb, vtiles = state[b]
                for j in range(NSTRIP):
                    stb = hpoolb.tile([128, STRIPW], bf16, tag="Hb")
                    nc.gpsimd.tensor_copy(stb[:], stripsf[j][:])
                    stripsb.append(stb)

            def emit_mm1_softmax(b):
                # mm1: S accumulated over 64 unit-pairs x (even, odd) x
                # (half0, half1).  Unit pair p covers units 2p, 2p+1:
                # 4 transposes into one PSUM bank, one batched copy, then
                # 8 accumulating [128c x 64 x 64] matmuls (2 per transpose,
                # one per n-half) all landing on the same [64,64] block.
                stripsf, stripsb, vtiles = state[b]
                s_ps = psS.tile([64, 64], f32, tag="S")
                to_sbs = {}
                NPAIR = NUNIT // 2

                def emit_transpose(pp):
                    to_ps = psT.tile([128, 4, 128], f32, tag="TO")
                    for q in range(2):
                        k = 2 * pp + q
                        j, tl = k // UPS, k % UPS
                        stv = stripsf[j].rearrange(
                            "p (a two) -> p a two", two=2
                        )
                        for ph in range(2):
                            nc.tensor.transpose(
                                to_ps[:, 2 * q + ph, :],
                                stv[:, tl * 128:(tl + 1) * 128, ph],
                                ident[:],
                            )
                    to_sb = topool.tile([128, 4, 128], f32, tag="TOs")
                    if pp % 2 == 0:
                        nc.scalar.copy(to_sb[:], to_ps[:])
                    else:
                        nc.vector.tensor_copy(to_sb[:], to_ps[:])
                    to_sbs[pp] = to_sb

                for pp in range(SKEW):
                    emit_transpose(pp)
                n_mm = 0
                for pp in range(NPAIR):
                    if pp + SKEW < NPAIR:
                        emit_transpose(pp + SKEW)
                    to_sb = to_sbs.pop(pp)
                    for q in range(2):
                        k = 2 * pp + q
                        j, tl = k // UPS, k % UPS
                        for ph in range(2):
                            for hh in range(2):
                                nc.tensor.matmul(
                                    s_ps[:],
                                    to_sb[:, 2 * q + ph,
                                          64 * hh:64 * hh + 64],
                                    vtiles[j][:, tl, hh,
                                              64 * ph:64 * ph + 64],
                                    start=(n_mm == 0),
                                    stop=(n_mm == 4 * NUNIT - 1),
                                )
                                n_mm += 1

                # softmax + (P + I), transposed into both partition halves,
                # cast to bf16 -- all reading S straight from PSUM.
                nmx = softpool.tile([64, 1], f32, tag="nmx")
                nc.vector.tensor_reduce(
                    nmx[:], s_ps[:], axis=mybir.AxisListType.X, op=Alu.max,
                    negate=True,
                )
                esum = softpool.tile([64, 1], f32, tag="esum")
                e_sb = softpool.tile([64, 64], f32, tag="E")
                nc.scalar.activation(
                    e_sb[:], s_ps[:], Act.Exp, bias=nmx[:, 0:1], scale=1.0,
                    accum_out=esum[:],
                )
                rcp = softpool.tile([64, 1], f32, tag="rcp")
                nc.vector.reciprocal(rcp[:], esum[:])
                pi_sb = softpool.tile([64, 64], f32, tag="PI")
                # PI = (E * 1/sum) + I
                nc.vector.scalar_tensor_tensor(
                    pi_sb[:], e_sb[:], rcp[:, 0:1], ident[0:64, 0:64],
                    Alu.mult, Alu.add,
                )
                pit_ps = psP.tile([64, 64], f32, tag="PIT")
                nc.tensor.transpose(
                    pit_ps[:], pi_sb[:], ident[0:64, 0:64])
                pit = softpool.tile([128, 64], bf16, tag="PITb")
                nc.vector.tensor_copy(pit[0:64, :], pit_ps[:])
                nc.sync.dma_start(pit[64:128, :], pit[0:64, :])
                return pit

            def emit_mm2(b, pit):
                # mm2 (bf16): out = (P+I) @ X1, 64 windows of 512 per half
                stripsf, stripsb, vtiles = state[b]
                o1 = ob[b].rearrange("(c n) -> c n", c=C)
                for half in range(2):
                    lhs = pit[64 * half:64 * half + 64, :]
                    for g in range(8):            # groups of 8 windows (4096)
                        stg = stpool.tile([128, 4, 512], bf16, tag="stage")
                        for wi in range(4):
                            o_ps = psO.tile([128, 512], f32, tag="O")
                            for hb in range(2):
                                stb = stripsb[2 * g + hb]
                                rhs = stb[64 * half:64 * half + 64,
                                          wi * 512:wi * 512 + 512]
                                nc.tensor.matmul(
                                    o_ps[64 * hb:64 * hb + 64, :], lhs, rhs,
                                    start=True, stop=True,
                                )
                            if wi % 2 == 0:
                                nc.vector.tensor_copy(
                                    stg[:, wi, :], o_ps[:]
                                )
                            else:
                                nc.scalar.copy(
                                    stg[:, wi, :], o_ps[:]
                                )
                        off = 32768 * half + g * 4096
                        nc.gpsimd.dma_start(
                            o1[:, off:off + 2048],
                            stg[0:64].rearrange("p a b -> p (a b)"),
                        )
                        nc.gpsimd.dma_start(
                            o1[:, off + 2048:off + 4096],
                            stg[64:128].rearrange("p a b -> p (a b)"),
                        )
                del state[b]

            loop_cm = (
                tc.For_i(0, loop_reps, 1) if loop_reps > 1 else nullcontext()
            )
            with loop_cm:
                # Software-pipelined emission: batch b+1's load DMAs are
                # enqueued on the rings BEFORE batch b's mm2/stores so the
                # DMA engines prefetch the next batch during mm2.
                emit_loads(0)
                emit_casts(0)
                for b in range(NB):
                    pit = emit_mm1_softmax(b)
                    if b + 1 < NB:
                        emit_loads(b + 1)
                    emit_mm2(b, pit)
                    if b + 1 < NB:
                        emit_casts(b + 1)

    nc.compile()
    return nc


def make_in_maps(x: np.ndarray) -> list:
    x = np.ascontiguousarray(x, dtype=np.float32)
    xflat = x.reshape(x.shape[0], -1)
    return [{"xb": xflat[NB * i:NB * (i + 1)]} for i in range(NCORES)]


def kernel(x: np.ndarray) -> np.ndarray:
    import ml_dtypes
    from concourse.bass_utils import run_bass_kernel_spmd

    if "nc" not in _CACHE:
        _CACHE["nc"] = _build()
    nc = _CACHE["nc"]

    x = np.ascontiguousarray(x, dtype=np.float32)
    B, Cc, H, W = x.shape
    xflat = x.reshape(B, Cc * H * W)
    in_maps = make_in_maps(x)
    res = run_bass_kernel_spmd(nc, in_maps, core_ids=list(range(NCORES)))
    out = np.empty_like(xflat)
    for i in range(NCORES):
        obv = res.results[i]["ob"]
        out[NB * i:NB * (i + 1)] = obv.view(ml_dtypes.bfloat16).astype(
            np.float32
        ) if obv.dtype != ml_dtypes.bfloat16 else obv.astype(np.float32)
    return out.reshape(B, Cc, H, W)
